# revision 5
# baseline (speedup 1.0000x reference)
"""AntiBiasL1Loss (segment_reduce over 5 grades) on 8 TRN2 NeuronCores.

Algorithm (same telescoped-matmul scheme as before):
  seg = round(y_true); e = |y_pred - y_true|
  For moving operands y, w_t = relu(y-t) (t=1..3), accumulate 4 matmuls
  psum_t += e_chunk.T @ mov_t over all [128,128] chunks.  Sentinel column
  per chunk (p=5, y=4 -> e=1) makes diag = masked segment sums, row 127 =
  weighted counts, col 127 of block 3 = sum(e).  Host un-telescopes.

Engine split (new vs. baseline):
  DVE    : m1,m2,m3 = relu(y-t)  (tensor_scalar dual, 4x) ; d = p - y (TT, 2x)
  ScalarE: e = |d| in place  (activation Abs)  -- was 2 DVE ops
  PE     : 4 accumulating matmuls per chunk into ONE [128,512] psum bank
  ScalarE: single [128,512] psum -> SBUF copy at the end (was 4 DVE copies)
  Pool   : y_true ships as fp8 (exact for integer grades) and is upcast
           fp8->bf16 by the SWDGE DMA itself (free); p ships bf16 on the
           sync HWDGE queue.  HBM traffic drops 8.5 -> 6.6 MB per core.

Single-wait discipline (each instruction encodes at most ONE sem wait):
  per tile the DVE order is m1 (waits y-DMA), m2, m3 (WAR on slot, RAW via
  m1), d (waits p-DMA; dst is a single-use slot), then ScalarE abs waits
  the DVE clock at d (which transitively covers the masks), so every
  matmul needs only the Scalar-clock wait.  m1/e tiles are single-use.

Startup surgery on the emitted BSP program (the first ~10.7us of the
baseline were engine bootstrap + barrier with DMA idle, and the PE ran
its first ~13us at the cold 1.2 GHz HAM clock):
  - the first HOIST input DMAs move before the init barrier, so data is
    in flight during bootstrap;
  - WARM dummy matmuls on a zeroed scratch tile slot in between the PE's
    barrier-arrival and barrier-wait, warming the HAM clock gate without
    delaying any other engine;
  - a dummy [128,1] activation slots in the same place on ScalarE so the
    one-time ~2.7us ACT table load happens during the barrier, not on the
    critical path;
  - the kernel-tail Drain keeps only its SWDGE (output DMA) wait.
"""

import numpy as np

import concourse.bass as bass
from concourse import mybir, tile
from concourse import tile_sem_assignment as _tsa
from concourse.bass_utils import run_bass_kernel_spmd

_tsa.NUM_SWDGE_GLOBAL_SEMS = 1
_tsa.NUM_HWDGE_SEMS = 1

P = 128
CORES = 8
N_TOTAL = 16_777_216
SHARD = N_TOTAL // CORES          # 2_097_152
FREE = SHARD // P                 # 16384 real columns per core
CHUNK = 128
REAL = CHUNK - 1
NCHUNK = -(-FREE // REAL)         # 130 chunks
TILES = (4, 13, 13, 13, 13, 13, 13, 13, 13, 13, 9)
NBF = 2          # first NBF tiles ship y as bf16 inside the combined tensor
WARM_MM = 20     # dummy matmuls to warm the PE HAM clock gate
HOIST = 4        # input DMAs moved before the init barrier
TOTC = NCHUNK * CHUNK
F32 = mybir.dt.float32
BF16 = mybir.dt.bfloat16
FP8 = mybir.dt.float8e4
assert sum(TILES) == NCHUNK and NCHUNK * REAL >= FREE

COMB_COLS = sum(2 * c * CHUNK for c in TILES[:NBF])
P_COLS = sum(c * CHUNK for c in TILES[NBF:])
Y8_COLS = P_COLS


def build_kernel(tiles=TILES):
    nc = bass.Bass(target_bir_lowering=False, debug=False)

    xin = nc.declare_dram_parameter("xin", [P, COMB_COLS + P_COLS], BF16, isOutput=False)
    xy8 = nc.declare_dram_parameter("xy8", [P, Y8_COLS], FP8, isOutput=False)
    out_ext = nc.declare_dram_parameter("out", [P, 4 * CHUNK], F32, isOutput=True)

    with tile.TileContext(nc) as tc:
        with (
            tc.tile_pool(name="inp", bufs=len(tiles)) as inp,      # single-use input slots
            tc.tile_pool(name="ypool", bufs=len(tiles) - NBF) as ypool,
            tc.tile_pool(name="epool", bufs=len(tiles)) as epool,  # single-use d/e slots
            tc.tile_pool(name="m1pool", bufs=len(tiles)) as m1pool,
            tc.tile_pool(name="mid", bufs=3) as mid,
            tc.tile_pool(name="stat", bufs=1) as stat,
            tc.tile_pool(name="scr", bufs=1) as scr,
            tc.tile_pool(name="psum", bufs=1, space=bass.MemorySpace.PSUM) as psum,
        ):
            # --- warmup block (relocated before/into the init barrier) ---
            wsrc = scr.tile([P, CHUNK], BF16, tag="wsrc", name="wsrc")
            nc.gpsimd.memset(wsrc[:, :], 0)
            wpsum = psum.tile([P, CHUNK], F32, tag="warm", name="warm")
            for _ in range(WARM_MM):
                nc.tensor.matmul(wpsum[:, :], wsrc[:, :], wsrc[:, :],
                                 start=True, stop=True)
            wact = scr.tile([P, 1], BF16, tag="wact", name="wact")
            nc.scalar.activation(wact[:, :], wsrc[:, :1],
                                 mybir.ActivationFunctionType.Abs)

            psum_t = psum.tile([P, 4 * CHUNK], F32, tag="ps", name="ps")

            off = 0       # column offset into xin
            yoff = 0      # column offset into xy8
            for j, cj in enumerate(tiles):
                tile_c = cj * CHUNK
                if j < NBF:
                    xt = inp.tile([P, 2 * tile_c], BF16, tag="xt", name=f"xt{j}")
                    nc.sync.dma_start(out=xt[:, :], in_=xin[:, off: off + 2 * tile_c])
                    off += 2 * tile_c
                    pt = xt[:, :tile_c]
                    yt = xt[:, tile_c:]
                else:
                    xt = inp.tile([P, tile_c], BF16, tag="xt", name=f"xt{j}")
                    nc.sync.dma_start(out=xt[:, :], in_=xin[:, off: off + tile_c])
                    off += tile_c
                    pt = xt[:, :]
                    ytile = ypool.tile([P, tile_c], BF16, tag="yt", name=f"yt{j}")
                    # SWDGE upcasts fp8 -> bf16 in the DMA datapath
                    nc.gpsimd.dma_start(out=ytile[:, :],
                                        in_=xy8[:, yoff: yoff + tile_c])
                    yoff += tile_c
                    yt = ytile[:, :]

                # masks first: m1 carries the y RAW wait (single-use slot, no
                # WAR); m2/m3 inherit the RAW via engine order and spend their
                # wait slot on the WAR against old matmul readers.
                m1 = m1pool.tile([P, tile_c], BF16, tag="m1")
                nc.vector.tensor_scalar(
                    m1[:, :], yt, 1.0, 0.0,
                    mybir.AluOpType.subtract, op1=mybir.AluOpType.max)
                masks = [yt, m1[:, :]]
                for t in (2.0, 3.0):
                    m = mid.tile([P, tile_c], BF16, tag=f"m{t}")
                    nc.vector.tensor_scalar(
                        m[:, :], yt, t, 0.0,
                        mybir.AluOpType.subtract, op1=mybir.AluOpType.max)
                    masks.append(m[:, :])

                # d = p - y into the fresh single-use e slot (waits p-DMA)
                e = epool.tile([P, tile_c], BF16, tag="e")
                nc.vector.tensor_tensor(e[:, :], pt, yt, mybir.AluOpType.subtract)
                # e = |d| in place on ScalarE; its DVE-clock wait at d
                # transitively covers the masks, so matmuls need 1 wait.
                nc.scalar.activation(e[:, :], e[:, :],
                                     mybir.ActivationFunctionType.Abs)

                for c in range(cj):
                    csl = slice(c * CHUNK, (c + 1) * CHUNK)
                    first = j == 0 and c == 0
                    last = j == len(tiles) - 1 and c == cj - 1
                    for t in range(4):
                        nc.tensor.matmul(
                            psum_t[:, t * CHUNK: (t + 1) * CHUNK],
                            e[:, csl],
                            masks[t][:, csl],
                            start=first,
                            stop=last,
                        )

            psum_sb = stat.tile([P, 4 * CHUNK], F32, tag="psb", name="psum_sb")
            nc.scalar.copy(psum_sb[:, :], psum_t[:, :])
            nc.gpsimd.dma_start(out=out_ext[:, :], in_=psum_sb[:, :])

    _surgery(nc)
    return nc


def _surgery(nc):
    """Post-hoc BSP program reordering + kernel-tail Drain patch."""
    blocks = nc.m.functions[0].blocks
    main, body = blocks[0], blocks[1]

    body_insts = list(body.instructions)
    # ---- identify relocatable startup instructions in the tile body ----
    scratch_memset = None
    warm = []            # warmup Ldweights/Matmult pairs
    dummy_act = []       # LoadActFuncSet + first (dummy) InstActivation
    hoist_dma = []       # first HOIST sync-queue input DMAs
    n_mm = 0
    for i in body_insts:
        tn = type(i).__name__
        if tn == "InstMemset" and scratch_memset is None:
            scratch_memset = i
        elif tn in ("InstLdweights", "InstMatmult") and n_mm < 2 * WARM_MM:
            warm.append(i)
            n_mm += 1
        elif tn in ("InstLoadActFuncSet", "InstActivation") and len(dummy_act) < 2:
            if tn == "InstActivation" and dummy_act and \
                    type(dummy_act[-1]).__name__ == "InstActivation":
                continue
            dummy_act.append(i)
        elif tn == "InstDMACopy" and str(i.engine) == "EngineType.SP" \
                and len(hoist_dma) < HOIST:
            hoist_dma.append(i)
    if dummy_act and type(dummy_act[0]).__name__ == "InstActivation":
        dummy_act = dummy_act[:1]

    moved = set(id(x) for x in ([scratch_memset] if scratch_memset else [])
                + warm + dummy_act + hoist_dma)
    body.instructions = [i for i in body_insts if id(i) not in moved]

    # ---- splice into the preamble block ----
    main_insts = list(main.instructions)
    first_drain = next(k for k, i in enumerate(main_insts)
                       if type(i).__name__ == "InstDrain")
    # before the barrier: hoisted DMAs (SP) + scratch memset (Pool)
    pre = hoist_dma + ([scratch_memset] if scratch_memset else [])
    main_insts[first_drain:first_drain] = pre

    def after_engine_drain(insts, engine_name, extra):
        for k, i in enumerate(insts):
            if type(i).__name__ == "InstDrain" and str(i.engine) == engine_name:
                return insts[:k + 1] + extra + insts[k + 1:]
        raise AssertionError(f"no drain for {engine_name}")

    # between barrier-arrival and barrier-wait: PE warmups, Scalar table load
    main_insts = after_engine_drain(main_insts, "EngineType.PE", warm)
    main_insts = after_engine_drain(main_insts, "EngineType.Activation", dummy_act)
    main.instructions = main_insts

    # ---- strip same-engine proc-clock waits (implied by FIFO order) ----
    # Tile sometimes emits a WAW wait on the instruction's own engine's
    # proc clock (e.g. a DVE op waiting DVE_nn>=k for the previous writer
    # of its pool slot).  In-order engine execution already guarantees
    # those; walrus rejects instructions with >1 encoded wait.
    eng_proc = {
        "EngineType.DVE": "DVE", "EngineType.PE": "PE",
        "EngineType.Activation": "Activation", "EngineType.Pool": "Pool",
        "EngineType.SP": "SP",
    }
    for b in nc.m.functions[0].blocks:
        for i in b.instructions:
            si = i.sync_info
            if not si or not si.on_wait or type(i).__name__ == "InstDrain":
                continue
            proc = eng_proc.get(str(getattr(i, "engine", None)))
            if proc is None:
                continue
            keep = [w for w in si.on_wait
                    if w.ant_name.rsplit("_", 1)[0] != proc]
            # a DMA's wait on its own queue's completion lane is implied by
            # per-queue FIFO (all increments come from earlier same-queue DMAs)
            if type(i).__name__ == "InstDMACopy" and len(keep) > 1:
                lane = "DMASW" if proc == "Pool" else "DMAHW"
                keep = [w for w in keep if not w.ant_name.startswith(lane)]
            if len(keep) != len(si.on_wait):
                i.sync_info = mybir.SyncInfo(on_wait=keep,
                                             on_update=list(si.on_update))

    # ---- kernel-tail Drain: keep only the output-DMA (SWDGE) wait ----
    for b in nc.m.functions[0].blocks:
        for i in b.instructions:
            si = i.sync_info
            if type(i).__name__ == "InstDrain" and si and len(si.on_wait) > 1:
                keep = [w for w in si.on_wait if w.ant_name.startswith("DMASW")]
                assert len(keep) == 1, [w.ant_name for w in si.on_wait]
                i.sync_info = mybir.SyncInfo(on_wait=keep,
                                             on_update=list(si.on_update))


def combine_outputs(outs, n_total: int = N_TOTAL) -> np.float32:
    """Host-side finish: un-telescope sums/counts, per-group means, mean."""
    v = np.zeros(4, np.float64)   # W, V1, V2, V3
    c = np.zeros(4, np.float64)   # Cy, D1, D2, D3
    sum_e = 0.0
    for o in outs:
        o = np.asarray(o, np.float64)
        for t in range(4):
            blk = o[:, t * CHUNK: (t + 1) * CHUNK]
            v[t] += np.trace(blk[:REAL, :REAL])
            c[t] += blk[REAL, :REAL].sum()
        sum_e += o[:, 3 * CHUNK: 4 * CHUNK][:REAL, REAL].sum()

    s_thr = np.array([v[0] - v[1], v[1] - v[2], v[2] - v[3], v[3]])
    c_thr = np.array([c[0] - c[1], c[1] - c[2], c[2] - c[3], c[3]])
    s_cum = np.array([sum_e, *s_thr, 0.0])
    c_cum = np.array([float(n_total), *c_thr, 0.0])
    sums = s_cum[:-1] - s_cum[1:]
    counts = c_cum[:-1] - c_cum[1:]
    present = counts > 0
    means = np.where(present, sums / np.where(present, counts, 1.0), 0.0)
    return np.float32(means.sum() / present.sum())


def pack_inputs(y_pred: np.ndarray, y_true: np.ndarray):
    """[N] f32 x2 -> per-core (xin bf16, xy8 fp8): sentinel col per chunk,
    zero-col padding; tiles 0..NBF-1 hold [p|y] bf16, later tiles ship p in
    xin and y in xy8 (fp8 is exact for integer grades)."""
    import ml_dtypes
    bf16 = np.dtype(ml_dtypes.bfloat16)
    fp8 = np.dtype(ml_dtypes.float8_e4m3)
    p = np.ascontiguousarray(y_pred, np.float32).reshape(CORES, P, FREE)
    y = np.ascontiguousarray(y_true, np.float32).reshape(CORES, P, FREE)
    pc = np.zeros((CORES, P, NCHUNK, CHUNK), np.float32)
    yc = np.zeros((CORES, P, NCHUNK, CHUNK), np.float32)
    tmp = np.zeros((CORES, P, NCHUNK * REAL), np.float32)
    tmp[:, :, :FREE] = p
    pc[:, :, :, :REAL] = tmp.reshape(CORES, P, NCHUNK, REAL)
    tmp[:, :, :FREE] = y
    yc[:, :, :, :REAL] = tmp.reshape(CORES, P, NCHUNK, REAL)
    pc[:, :, :, REAL] = 5.0
    yc[:, :, :, REAL] = 4.0
    pc = pc.reshape(CORES, P, TOTC).astype(bf16)
    yc = yc.reshape(CORES, P, TOTC)

    xin = np.empty((CORES, P, COMB_COLS + P_COLS), bf16)
    xy8 = np.empty((CORES, P, Y8_COLS), fp8)
    off = 0
    coff = 0
    for j, cj in enumerate(TILES):
        t = cj * CHUNK
        if j < NBF:
            xin[:, :, off: off + t] = pc[:, :, coff: coff + t]
            xin[:, :, off + t: off + 2 * t] = yc[:, :, coff: coff + t].astype(bf16)
            off += 2 * t
        else:
            xin[:, :, off: off + t] = pc[:, :, coff: coff + t]
            off += t
        coff += t
    coff = sum(c * CHUNK for c in TILES[:NBF])
    xy8[:, :, :] = yc[:, :, coff:].astype(fp8)
    return xin, xy8


def run(y_pred: np.ndarray, y_true: np.ndarray, trace: bool = False, **kw):
    xin, xy8 = pack_inputs(y_pred, y_true)
    in_maps = [{"xin": xin[i], "xy8": xy8[i]} for i in range(CORES)]
    nc = build_kernel()
    res = run_bass_kernel_spmd(
        nc, in_maps, core_ids=list(range(CORES)), trace=trace, **kw
    )
    outs = [res.results[i]["out"] for i in range(CORES)]
    return np.asarray(combine_outputs(outs), np.float32), res


def kernel(y_pred: np.ndarray, y_true: np.ndarray) -> np.ndarray:
    return run(y_pred, y_true)[0]


# revision 7
# speedup vs baseline: 1.3135x; 1.3135x over previous
"""AntiBiasL1Loss (segment_reduce over 5 grades) on 8 TRN2 NeuronCores.

Algorithm (same telescoped-matmul scheme as before):
  seg = round(y_true); e = |y_pred - y_true|
  For moving operands y, w_t = relu(y-t) (t=1..3), accumulate 4 matmuls
  psum_t += e_chunk.T @ mov_t over all [128,128] chunks.  Sentinel column
  per chunk (p=5, y=4 -> e=1) makes diag = masked segment sums, row 127 =
  weighted counts, col 127 of block 3 = sum(e).  Host un-telescopes.

Engine split (new vs. baseline):
  DVE    : m1,m2,m3 = relu(y-t)  (tensor_scalar dual, 4x) ; d = p - y (TT, 2x)
  ScalarE: e = |d| in place  (activation Abs)  -- was 2 DVE ops
  PE     : 4 accumulating matmuls per chunk into ONE [128,512] psum bank
  ScalarE: single [128,512] psum -> SBUF copy at the end (was 4 DVE copies)
  Pool   : y_true ships as fp8 (exact for integer grades) and is upcast
           fp8->bf16 by the SWDGE DMA itself (free); p ships bf16 on the
           sync HWDGE queue.  HBM traffic drops 8.5 -> 6.6 MB per core.

Single-wait discipline (each instruction encodes at most ONE sem wait):
  per tile the DVE order is m1 (waits y-DMA), m2, m3 (WAR on slot, RAW via
  m1), d (waits p-DMA; dst is a single-use slot), then ScalarE abs waits
  the DVE clock at d (which transitively covers the masks), so every
  matmul needs only the Scalar-clock wait.  m1/e tiles are single-use.

Startup surgery on the emitted BSP program (the first ~10.7us of the
baseline were engine bootstrap + barrier with DMA idle, and the PE ran
its first ~13us at the cold 1.2 GHz HAM clock):
  - the first HOIST input DMAs move before the init barrier, so data is
    in flight during bootstrap;
  - WARM dummy matmuls on a zeroed scratch tile slot in between the PE's
    barrier-arrival and barrier-wait, warming the HAM clock gate without
    delaying any other engine;
  - a dummy [128,1] activation slots in the same place on ScalarE so the
    one-time ~2.7us ACT table load happens during the barrier, not on the
    critical path;
  - the kernel-tail Drain keeps only its SWDGE (output DMA) wait.
"""

import numpy as np

import concourse.bass as bass
from concourse import mybir, tile
from concourse import tile_sem_assignment as _tsa
from concourse.bass_utils import run_bass_kernel_spmd

_tsa.NUM_SWDGE_GLOBAL_SEMS = 1
_tsa.NUM_HWDGE_SEMS = 1

P = 128
CORES = 8
N_TOTAL = 16_777_216
SHARD = N_TOTAL // CORES          # 2_097_152
FREE = SHARD // P                 # 16384 real columns per core
CHUNK = 128
REAL = CHUNK - 1
NCHUNK = -(-FREE // REAL)         # 130 chunks
TILES = (4, 13, 13, 13, 13, 13, 13, 13, 13, 13, 9)
NBF = 2          # first NBF tiles ship y as bf16 inside the combined tensor
WARM_MM = 45     # dummy matmuls to warm the PE HAM clock gate
HOIST = 4        # input DMAs moved before the init barrier
TOTC = NCHUNK * CHUNK
F32 = mybir.dt.float32
BF16 = mybir.dt.bfloat16
FP8 = mybir.dt.float8e4
assert sum(TILES) == NCHUNK and NCHUNK * REAL >= FREE

COMB_COLS = sum(2 * c * CHUNK for c in TILES[:NBF])
P_COLS = sum(c * CHUNK for c in TILES[NBF:])
Y8_COLS = P_COLS


def build_kernel(tiles=TILES):
    nc = bass.Bass(target_bir_lowering=False, debug=False)

    xin = nc.declare_dram_parameter("xin", [P, COMB_COLS + P_COLS], BF16, isOutput=False)
    xy8 = nc.declare_dram_parameter("xy8", [P, Y8_COLS], FP8, isOutput=False)
    out_ext = nc.declare_dram_parameter("out", [P, 4 * CHUNK], F32, isOutput=True)

    with tile.TileContext(nc) as tc:
        with (
            tc.tile_pool(name="inp", bufs=len(tiles)) as inp,      # single-use input slots
            tc.tile_pool(name="ypool", bufs=len(tiles) - NBF) as ypool,
            tc.tile_pool(name="epool", bufs=len(tiles)) as epool,  # single-use d/e slots
            tc.tile_pool(name="m1pool", bufs=len(tiles)) as m1pool,
            tc.tile_pool(name="mid", bufs=3) as mid,
            tc.tile_pool(name="stat", bufs=1) as stat,
            tc.tile_pool(name="scr", bufs=1) as scr,
            tc.tile_pool(name="psum", bufs=1, space=bass.MemorySpace.PSUM) as psum,
        ):
            # --- warmup block (relocated before/into the init barrier) ---
            wsrc = scr.tile([P, CHUNK], BF16, tag="wsrc", name="wsrc")
            nc.gpsimd.memset(wsrc[:, :], 0)
            wpsum = psum.tile([P, CHUNK], F32, tag="warm", name="warm")
            for _ in range(WARM_MM):
                nc.tensor.matmul(wpsum[:, :], wsrc[:, :], wsrc[:, :],
                                 start=True, stop=True)
            wact = scr.tile([P, 1], BF16, tag="wact", name="wact")
            nc.scalar.activation(wact[:, :], wsrc[:, :1],
                                 mybir.ActivationFunctionType.Abs)

            psum_t = psum.tile([P, 4 * CHUNK], F32, tag="ps", name="ps")

            off = 0       # column offset into xin
            yoff = 0      # column offset into xy8
            for j, cj in enumerate(tiles):
                tile_c = cj * CHUNK
                if j < NBF:
                    xt = inp.tile([P, 2 * tile_c], BF16, tag="xt", name=f"xt{j}")
                    nc.sync.dma_start(out=xt[:, :], in_=xin[:, off: off + 2 * tile_c])
                    off += 2 * tile_c
                    pt = xt[:, :tile_c]
                    yt = xt[:, tile_c:]
                else:
                    xt = inp.tile([P, tile_c], BF16, tag="xt", name=f"xt{j}")
                    nc.sync.dma_start(out=xt[:, :], in_=xin[:, off: off + tile_c])
                    off += tile_c
                    pt = xt[:, :]
                    ytile = ypool.tile([P, tile_c], BF16, tag="yt", name=f"yt{j}")
                    # SWDGE upcasts fp8 -> bf16 in the DMA datapath
                    nc.gpsimd.dma_start(out=ytile[:, :],
                                        in_=xy8[:, yoff: yoff + tile_c])
                    yoff += tile_c
                    yt = ytile[:, :]

                # masks first: m1 carries the y RAW wait (single-use slot, no
                # WAR); m2/m3 inherit the RAW via engine order and spend their
                # wait slot on the WAR against old matmul readers.
                m1 = m1pool.tile([P, tile_c], BF16, tag="m1")
                nc.vector.tensor_scalar(
                    m1[:, :], yt, 1.0, 0.0,
                    mybir.AluOpType.subtract, op1=mybir.AluOpType.max)
                masks = [yt, m1[:, :]]
                for t in (2.0, 3.0):
                    m = mid.tile([P, tile_c], BF16, tag=f"m{t}")
                    nc.vector.tensor_scalar(
                        m[:, :], yt, t, 0.0,
                        mybir.AluOpType.subtract, op1=mybir.AluOpType.max)
                    masks.append(m[:, :])

                # d = p - y into the fresh single-use e slot (waits p-DMA)
                e = epool.tile([P, tile_c], BF16, tag="e")
                nc.vector.tensor_tensor(e[:, :], pt, yt, mybir.AluOpType.subtract)
                # e = |d| in place on ScalarE; its DVE-clock wait at d
                # transitively covers the masks, so matmuls need 1 wait.
                nc.scalar.activation(e[:, :], e[:, :],
                                     mybir.ActivationFunctionType.Abs)

                for c in range(cj):
                    csl = slice(c * CHUNK, (c + 1) * CHUNK)
                    first = j == 0 and c == 0
                    last = j == len(tiles) - 1 and c == cj - 1
                    for t in range(4):
                        nc.tensor.matmul(
                            psum_t[:, t * CHUNK: (t + 1) * CHUNK],
                            e[:, csl],
                            masks[t][:, csl],
                            start=first,
                            stop=last,
                        )

            psum_sb = stat.tile([P, 4 * CHUNK], F32, tag="psb", name="psum_sb")
            nc.scalar.copy(psum_sb[:, :], psum_t[:, :])
            nc.gpsimd.dma_start(out=out_ext[:, :], in_=psum_sb[:, :])

    _surgery(nc)
    return nc


def _surgery(nc):
    """Post-hoc BSP program reordering + kernel-tail Drain patch."""
    blocks = nc.m.functions[0].blocks
    main, body = blocks[0], blocks[1]

    body_insts = list(body.instructions)
    # ---- identify relocatable startup instructions in the tile body ----
    scratch_memset = None
    warm = []            # warmup Ldweights/Matmult pairs
    dummy_act = []       # LoadActFuncSet + first (dummy) InstActivation
    hoist_dma = []       # first HOIST sync-queue input DMAs
    n_mm = 0
    for i in body_insts:
        tn = type(i).__name__
        if tn == "InstMemset" and scratch_memset is None:
            scratch_memset = i
        elif tn in ("InstLdweights", "InstMatmult") and n_mm < 2 * WARM_MM:
            warm.append(i)
            n_mm += 1
        elif tn in ("InstLoadActFuncSet", "InstActivation") and len(dummy_act) < 2:
            if tn == "InstActivation" and dummy_act and \
                    type(dummy_act[-1]).__name__ == "InstActivation":
                continue
            dummy_act.append(i)
        elif tn == "InstDMACopy" and str(i.engine) == "EngineType.SP" \
                and len(hoist_dma) < HOIST:
            hoist_dma.append(i)
    if dummy_act and type(dummy_act[0]).__name__ == "InstActivation":
        dummy_act = dummy_act[:1]

    moved = set(id(x) for x in ([scratch_memset] if scratch_memset else [])
                + warm + dummy_act + hoist_dma)
    body.instructions = [i for i in body_insts if id(i) not in moved]

    # ---- splice into the preamble block ----
    main_insts = list(main.instructions)
    first_drain = next(k for k, i in enumerate(main_insts)
                       if type(i).__name__ == "InstDrain")
    # before the barrier: hoisted DMAs (SP) + scratch memset (Pool)
    pre = hoist_dma + ([scratch_memset] if scratch_memset else [])
    main_insts[first_drain:first_drain] = pre

    def after_engine_drain(insts, engine_name, extra):
        for k, i in enumerate(insts):
            if type(i).__name__ == "InstDrain" and str(i.engine) == engine_name:
                return insts[:k + 1] + extra + insts[k + 1:]
        raise AssertionError(f"no drain for {engine_name}")

    # between barrier-arrival and barrier-wait: PE warmups, Scalar table load
    main_insts = after_engine_drain(main_insts, "EngineType.PE", warm)
    main_insts = after_engine_drain(main_insts, "EngineType.Activation", dummy_act)
    main.instructions = main_insts

    # ---- strip same-engine proc-clock waits (implied by FIFO order) ----
    # Tile sometimes emits a WAW wait on the instruction's own engine's
    # proc clock (e.g. a DVE op waiting DVE_nn>=k for the previous writer
    # of its pool slot).  In-order engine execution already guarantees
    # those; walrus rejects instructions with >1 encoded wait.
    eng_proc = {
        "EngineType.DVE": "DVE", "EngineType.PE": "PE",
        "EngineType.Activation": "Activation", "EngineType.Pool": "Pool",
        "EngineType.SP": "SP",
    }
    for b in nc.m.functions[0].blocks:
        for i in b.instructions:
            si = i.sync_info
            if not si or not si.on_wait or type(i).__name__ == "InstDrain":
                continue
            proc = eng_proc.get(str(getattr(i, "engine", None)))
            if proc is None:
                continue
            keep = [w for w in si.on_wait
                    if w.ant_name.rsplit("_", 1)[0] != proc]
            # a DMA's wait on its own queue's completion lane is implied by
            # per-queue FIFO (all increments come from earlier same-queue
            # DMAs, and every dest slot here is single-use).  Stripping it
            # un-serializes the input stream: Tile otherwise makes DMA j+1
            # wait for DMA j's last byte before even issuing.
            if type(i).__name__ == "InstDMACopy":
                lane = "DMASW" if proc == "Pool" else "DMAHW"
                keep = [w for w in keep if not w.ant_name.startswith(lane)]
            if len(keep) != len(si.on_wait):
                i.sync_info = mybir.SyncInfo(on_wait=keep,
                                             on_update=list(si.on_update))

    # ---- kernel-tail Drain: keep only the output-DMA (SWDGE) wait ----
    for b in nc.m.functions[0].blocks:
        for i in b.instructions:
            si = i.sync_info
            if type(i).__name__ == "InstDrain" and si and len(si.on_wait) > 1:
                keep = [w for w in si.on_wait if w.ant_name.startswith("DMASW")]
                assert len(keep) == 1, [w.ant_name for w in si.on_wait]
                i.sync_info = mybir.SyncInfo(on_wait=keep,
                                             on_update=list(si.on_update))


def combine_outputs(outs, n_total: int = N_TOTAL) -> np.float32:
    """Host-side finish: un-telescope sums/counts, per-group means, mean."""
    v = np.zeros(4, np.float64)   # W, V1, V2, V3
    c = np.zeros(4, np.float64)   # Cy, D1, D2, D3
    sum_e = 0.0
    for o in outs:
        o = np.asarray(o, np.float64)
        for t in range(4):
            blk = o[:, t * CHUNK: (t + 1) * CHUNK]
            v[t] += np.trace(blk[:REAL, :REAL])
            c[t] += blk[REAL, :REAL].sum()
        sum_e += o[:, 3 * CHUNK: 4 * CHUNK][:REAL, REAL].sum()

    s_thr = np.array([v[0] - v[1], v[1] - v[2], v[2] - v[3], v[3]])
    c_thr = np.array([c[0] - c[1], c[1] - c[2], c[2] - c[3], c[3]])
    s_cum = np.array([sum_e, *s_thr, 0.0])
    c_cum = np.array([float(n_total), *c_thr, 0.0])
    sums = s_cum[:-1] - s_cum[1:]
    counts = c_cum[:-1] - c_cum[1:]
    present = counts > 0
    means = np.where(present, sums / np.where(present, counts, 1.0), 0.0)
    return np.float32(means.sum() / present.sum())


def pack_inputs(y_pred: np.ndarray, y_true: np.ndarray):
    """[N] f32 x2 -> per-core (xin bf16, xy8 fp8): sentinel col per chunk,
    zero-col padding; tiles 0..NBF-1 hold [p|y] bf16, later tiles ship p in
    xin and y in xy8 (fp8 is exact for integer grades)."""
    import ml_dtypes
    bf16 = np.dtype(ml_dtypes.bfloat16)
    fp8 = np.dtype(ml_dtypes.float8_e4m3)
    p = np.ascontiguousarray(y_pred, np.float32).reshape(CORES, P, FREE)
    y = np.ascontiguousarray(y_true, np.float32).reshape(CORES, P, FREE)
    pc = np.zeros((CORES, P, NCHUNK, CHUNK), np.float32)
    yc = np.zeros((CORES, P, NCHUNK, CHUNK), np.float32)
    tmp = np.zeros((CORES, P, NCHUNK * REAL), np.float32)
    tmp[:, :, :FREE] = p
    pc[:, :, :, :REAL] = tmp.reshape(CORES, P, NCHUNK, REAL)
    tmp[:, :, :FREE] = y
    yc[:, :, :, :REAL] = tmp.reshape(CORES, P, NCHUNK, REAL)
    pc[:, :, :, REAL] = 5.0
    yc[:, :, :, REAL] = 4.0
    pc = pc.reshape(CORES, P, TOTC).astype(bf16)
    yc = yc.reshape(CORES, P, TOTC)

    xin = np.empty((CORES, P, COMB_COLS + P_COLS), bf16)
    xy8 = np.empty((CORES, P, Y8_COLS), fp8)
    off = 0
    coff = 0
    for j, cj in enumerate(TILES):
        t = cj * CHUNK
        if j < NBF:
            xin[:, :, off: off + t] = pc[:, :, coff: coff + t]
            xin[:, :, off + t: off + 2 * t] = yc[:, :, coff: coff + t].astype(bf16)
            off += 2 * t
        else:
            xin[:, :, off: off + t] = pc[:, :, coff: coff + t]
            off += t
        coff += t
    coff = sum(c * CHUNK for c in TILES[:NBF])
    xy8[:, :, :] = yc[:, :, coff:].astype(fp8)
    return xin, xy8


def run(y_pred: np.ndarray, y_true: np.ndarray, trace: bool = False, **kw):
    xin, xy8 = pack_inputs(y_pred, y_true)
    in_maps = [{"xin": xin[i], "xy8": xy8[i]} for i in range(CORES)]
    nc = build_kernel()
    res = run_bass_kernel_spmd(
        nc, in_maps, core_ids=list(range(CORES)), trace=trace, **kw
    )
    outs = [res.results[i]["out"] for i in range(CORES)]
    return np.asarray(combine_outputs(outs), np.float32), res


def kernel(y_pred: np.ndarray, y_true: np.ndarray) -> np.ndarray:
    return run(y_pred, y_true)[0]
